# revision 11
# baseline (speedup 1.0000x reference)
"""Trainium2 Bass kernel for nn_MoEDiscriminator (8 experts, MLP 64->256->256->1).

Strategy (data-parallel over 8 NeuronCores):
- st [65536, 64] is sharded along batch: 8192 rows per core; expert weights
  are replicated on every core.
- All computation runs in a "transposed" layout: activations live as
  [feature_on_partitions, batch_on_free] SBUF tiles, so every matmul streams
  batch columns through the PE array with the (small) weights stationary.
- float32r matmuls: full-fp32 operands at 1 cycle/row for free-dim >= 256.
- Per expert c, per batch tile of 512:
    L1: h1T[half] [128, 512] = W1[c,half]^T @ stT           (2 matmuls, K padded to 128)
    evict: relu(x + b1) PSUM->SBUF on ScalarE/VectorE
    L2: h2T[half] [128, 512] = sum_k W2[c,k,half]^T @ h1T[k]  (4 matmuls)
    evict: relu(x + b2)
    L3: d [8, 512] += W3pad[c,k]^T @ h2T[k]   (2 matmuls; W3pad holds the
        expert's weights in column c, zeros elsewhere, so all 8 experts
        accumulate into one [8, 512] PSUM tile)
- b3 is added on the host; output is reassembled host-side to [65536, 8, 1].
"""

import sys

sys.path.insert(0, "/opt/trn_rl_repo")
from contextlib import ExitStack

import numpy as np

import concourse.bass as bass
import concourse.tile as tile
from concourse import bacc, mybir
from concourse.bass import ts
from concourse.bass_utils import run_bass_kernel_spmd

P = 128
C = 8            # experts
DS = 64          # input feature dim
H = 256          # hidden width
B = 65536        # full batch
NCORES = 8
NB = B // NCORES  # 8192 rows per core
BT = 512         # batch tile (free dim of matmuls)
NT = NB // BT    # 16
ST_CHUNKS = (512, 1536, 2048, 4096)   # graduated st chunk widths
PSUM_BUFS = (4, 3, 1)                 # (psumA, psumB, psumD)
WORK_BUFS = 4

f32 = mybir.dt.float32
f32r = mybir.dt.float32r
AF = mybir.ActivationFunctionType
ALU = mybir.AluOpType

_NC_CACHE = {}


def _build_nc(repeats=1, hw_loop=False):
    key = (repeats, hw_loop)
    if key in _NC_CACHE:
        return _NC_CACHE[key]
    nc = bacc.Bacc("TRN2", target_bir_lowering=False, debug=False,
                   num_devices=NCORES)
    st_d = nc.dram_tensor("st", [P, NB], f32r, kind="ExternalInput").ap()
    w1_d = nc.dram_tensor("w1", [C, 2, P, P], f32r, kind="ExternalInput").ap()
    w2_d = nc.dram_tensor("w2", [C, 2, 2, P, P], f32r, kind="ExternalInput").ap()
    w3_d = nc.dram_tensor("w3", [C, 2, P, C], f32r, kind="ExternalInput").ap()
    b1_d = nc.dram_tensor("b1", [P, C * 2], f32, kind="ExternalInput").ap()
    b2_d = nc.dram_tensor("b2", [P, C * 2], f32, kind="ExternalInput").ap()
    d_d = nc.dram_tensor("d", [C, NB], f32, kind="ExternalOutput").ap()

    with tile.TileContext(nc) as tc, ExitStack() as ctx:
        const = ctx.enter_context(tc.tile_pool(name="const", bufs=2))
        work = ctx.enter_context(tc.tile_pool(name="work", bufs=WORK_BUFS))
        psumA = ctx.enter_context(
            tc.tile_pool(name="psumA", bufs=PSUM_BUFS[0], space="PSUM"))
        psumB = ctx.enter_context(
            tc.tile_pool(name="psumB", bufs=PSUM_BUFS[1], space="PSUM"))
        psumD = ctx.enter_context(
            tc.tile_pool(name="psumD", bufs=PSUM_BUFS[2], space="PSUM"))

        def body():
            # Small constants first so compute can start as early as possible.
            w1_sb = const.tile([P, C, 2, P], f32r)
            nc.sync.dma_start(w1_sb[:, 0:1],
                              w1_d[0:1].rearrange("c h p f -> p c h f"))
            b1_sb = const.tile([P, C * 2], f32)
            nc.sync.dma_start(b1_sb[:], b1_d)
            nc.sync.dma_start(w1_sb[:, 1:C],
                              w1_d[1:C].rearrange("c h p f -> p c h f"))

            # st in graduated chunks: a tiny first chunk lets the first batch
            # tile start almost immediately; W2 (2MB) lands mid-stream.
            st_sb = []
            st_off = []
            off = 0
            for i, cols in enumerate(ST_CHUNKS):
                t_ = const.tile([P, cols], f32r, name=f"st_sb{i}")
                st_sb.append(t_)
                st_off.append(off)
                off += cols
            assert off == NB
            nc.sync.dma_start(st_sb[0][:], st_d[:, 0:ST_CHUNKS[0]])
            b2_sb = const.tile([P, C * 2], f32)
            nc.sync.dma_start(b2_sb[:], b2_d)
            w3_sb = const.tile([P, C, 2, C], f32r)
            nc.sync.dma_start(w3_sb[:], w3_d.rearrange("c k p f -> p c k f"))
            nc.sync.dma_start(st_sb[1][:],
                              st_d[:, st_off[1]:st_off[1] + ST_CHUNKS[1]])
            w2_sb = const.tile([P, C, 2, 2, P], f32r)
            nc.sync.dma_start(w2_sb[:, 0:4],
                              w2_d[0:4].rearrange("c k j p f -> p c k j f"))
            nc.sync.dma_start(w2_sb[:, 4:8],
                              w2_d[4:8].rearrange("c k j p f -> p c k j f"))
            for i in range(2, len(ST_CHUNKS)):
                nc.sync.dma_start(st_sb[i][:],
                                  st_d[:, st_off[i]:st_off[i] + ST_CHUNKS[i]])

            d_sb = const.tile([C, NB], f32)

            def st_slice(t):
                col = t * BT
                for i, o in enumerate(st_off):
                    if o <= col < o + ST_CHUNKS[i]:
                        return st_sb[i][:, col - o:col - o + BT]
                raise AssertionError

            for t in range(NT):
                st_t = st_slice(t)
                pD = psumD.tile([C, BT], f32)
                for c in range(C):
                    # ---- L1 ----
                    pA = [psumA.tile([P, BT], f32, tag="pA", name=f"pA{h}")
                          for h in range(2)]
                    for h in range(2):
                        nc.tensor.matmul(
                            pA[h][:],
                            w1_sb[:, c, h, :],
                            st_t,
                            start=True, stop=True,
                        )
                    h1 = [work.tile([P, BT], f32r, tag="h1", name=f"h1_{h}")
                          for h in range(2)]
                    ea, ev = (0, 1) if c % 2 == 0 else (1, 0)
                    nc.scalar.activation(h1[ea][:], pA[ea][:], AF.Relu,
                                         bias=b1_sb[:, 2 * c + ea:2 * c + ea + 1])
                    nc.vector.tensor_scalar(h1[ev][:], pA[ev][:],
                                            b1_sb[:, 2 * c + ev:2 * c + ev + 1], 0.0,
                                            ALU.add, ALU.max)
                    # ---- L2 ----
                    pB = [psumB.tile([P, BT], f32, tag="pB", name=f"pB{h}")
                          for h in range(2)]
                    for j in range(2):
                        for k in range(2):
                            nc.tensor.matmul(
                                pB[j][:],
                                w2_sb[:, c, k, j, :],
                                h1[k][:],
                                start=(k == 0), stop=(k == 1),
                            )
                    h2 = [work.tile([P, BT], f32r, tag="h2", name=f"h2_{h}")
                          for h in range(2)]
                    nc.scalar.activation(h2[ea][:], pB[ea][:], AF.Relu,
                                         bias=b2_sb[:, 2 * c + ea:2 * c + ea + 1])
                    nc.vector.tensor_scalar(h2[ev][:], pB[ev][:],
                                            b2_sb[:, 2 * c + ev:2 * c + ev + 1], 0.0,
                                            ALU.add, ALU.max)
                    # ---- L3 (all experts accumulate into one [C, BT] psum) ----
                    for k in range(2):
                        nc.tensor.matmul(
                            pD[:],
                            w3_sb[:, c, k, :],
                            h2[k][:],
                            start=(c == 0 and k == 0),
                            stop=(c == C - 1 and k == 1),
                        )
                nc.vector.tensor_copy(d_sb[:, ts(t, BT)], pD[:])
            nc.sync.dma_start(d_d, d_sb[:])

        if hw_loop and repeats > 1:
            with tc.For_i(0, repeats, 1):
                body()
        else:
            for _rep in range(repeats):
                body()

    nc.compile()
    _NC_CACHE[key] = nc
    return nc


def _prep_weights(W1, b1, W2, b2, W3):
    W1p = np.zeros((C, 2, P, P), np.float32)
    for c in range(C):
        for h in range(2):
            W1p[c, h, :DS, :] = W1[c][:, h * P:(h + 1) * P]
    W2r = np.ascontiguousarray(
        W2.reshape(C, 2, P, 2, P).transpose(0, 1, 3, 2, 4)
    )  # W2r[c,k,j] = W2[c, k*128:(k+1)*128, j*128:(j+1)*128]
    W3p = np.zeros((C, 2, P, C), np.float32)
    for c in range(C):
        for k in range(2):
            W3p[c, k, :, c] = W3[c, k * P:(k + 1) * P, 0]
    b1h = np.ascontiguousarray(b1.reshape(C * 2, P).T)  # [128, C*2]
    b2h = np.ascontiguousarray(b2.reshape(C * 2, P).T)
    return W1p, W2r, W3p, b1h, b2h


def _make_in_maps(st, W1, b1, W2, b2, W3):
    W1p, W2r, W3p, b1h, b2h = _prep_weights(W1, b1, W2, b2, W3)
    in_maps = []
    for core in range(NCORES):
        shard = st[core * NB:(core + 1) * NB]            # [8192, 64]
        stT = np.ascontiguousarray(
            np.concatenate([shard.T, shard.T], axis=0))   # [128, 8192]
        in_maps.append({
            "st": stT, "w1": W1p, "w2": W2r, "w3": W3p,
            "b1": b1h, "b2": b2h,
        })
    return in_maps


def kernel(st, W1, b1, W2, b2, W3, b3):
    st = np.ascontiguousarray(np.asarray(st, np.float32))
    in_maps = _make_in_maps(
        st,
        np.asarray(W1, np.float32), np.asarray(b1, np.float32),
        np.asarray(W2, np.float32), np.asarray(b2, np.float32),
        np.asarray(W3, np.float32))
    nc = _build_nc(1)
    res = run_bass_kernel_spmd(nc, in_maps, core_ids=list(range(NCORES)))

    b3v = np.asarray(b3, np.float32).reshape(1, C)
    out = np.empty((B, C, 1), np.float32)
    for core in range(NCORES):
        d = res.results[core]["d"]                        # [8, 8192]
        out[core * NB:(core + 1) * NB, :, 0] = d.T + b3v
    return out


def bench(inputs, repeats, trials=3):
    """Wall-clock one spmd execution of the `repeats`-times-unrolled program.
    Returns min seconds over trials (first call pays the neuronx compile)."""
    import time

    in_maps = _make_in_maps(
        np.ascontiguousarray(np.asarray(inputs["st"], np.float32)),
        np.asarray(inputs["W1"], np.float32), np.asarray(inputs["b1"], np.float32),
        np.asarray(inputs["W2"], np.float32), np.asarray(inputs["b2"], np.float32),
        np.asarray(inputs["W3"], np.float32))
    nc = _build_nc(repeats)
    run_bass_kernel_spmd(nc, in_maps, core_ids=list(range(NCORES)))  # warm-up/compile
    best = float("inf")
    for _ in range(trials):
        t0 = time.perf_counter()
        run_bass_kernel_spmd(nc, in_maps, core_ids=list(range(NCORES)))
        best = min(best, time.perf_counter() - t0)
    return best
